# revision 1
# baseline (speedup 1.0000x reference)
"""Valid 3x3x3 conv3d: x[2,32,64,64,64] (*) W[64,32,3,3,3] -> y[2,64,62,62,62].

Sharding: D axis split across 8 cores (8 output planes each, 2-plane input halo,
sliced host-side). Batch = 2 independent streams per core (double-buffered SBUF
slots) so plane DMA+rounding overlaps PE compute of the other stream.

Per-core compute: conv as 27 shifted matmuls reduced to 6 per 8-row block:
  - K = 96: in_c(32) x kz(3); plane d lives at partition group (d mod 3), so the
    3 kz taps of any output plane occupy disjoint partition groups with no data
    replication. Weight column layout is rotated per (output plane mod 3).
  - kx taps 0,1 are paired into one M=128 matmul (rows 0:64 aligned, rows
    64:128 misaligned by +1 output column, fixed in the ACT+DVE combine); kx=2
    is an aligned M=64 matmul. 3 ky taps -> 3 pairs + 3 singles, all
    accumulating into one PSUM bank per 8-row block (N = nh*64 <= 512).
  - fp32r matmuls (1 col/cycle at N>=256, ~1.4e-4 rel err); inputs DMA straight
    into fp32r SBUF tiles (walrus accepts DMACopy as the fp32r producer; all
    fp32r APs must have even innermost counts/strides and dst partition 0).
"""
import sys
sys.path.insert(0, '/opt/trn_rl_repo')
import numpy as np

IN_C, OUT_C = 32, 64
SH = SW = 64
OD = 62
PD = 8          # output planes per core per batch
HALO = 2
NB = 2          # batches/streams
BLOCKS = [(h0, 8 if h0 + 8 <= OD else OD - h0) for h0 in range(0, OD, 8)]

_cache = {}


def _build():
    import concourse.bacc as bacc
    import concourse.mybir as mybir
    from concourse import tile
    dt = mybir.dt

    nc = bacc.Bacc(trn_type="TRN2")
    x_d = nc.declare_dram_parameter("x", [NB, IN_C, PD + HALO, SH * SW],
                                    dt.float32, isOutput=False)
    w_d = nc.declare_dram_parameter("w", [96, 3, 3, 192], dt.float32,
                                    isOutput=False)
    y_d = nc.declare_dram_parameter("y", [NB, OUT_C, PD, OD, OD], dt.float32,
                                    isOutput=True)

    with tile.TileContext(nc) as tc:
        with tc.tile_pool(name="xb", bufs=1) as xb_pool, \
             tc.tile_pool(name="wb", bufs=1) as wb_pool, \
             tc.tile_pool(name="ps", bufs=4, space="PSUM") as ps_pool, \
             tc.tile_pool(name="ob", bufs=4) as ob_pool:

            # weights: DMA straight into fp32r (bit-identical; walrus accepts
            # DMACopy as an fp32r producer)
            wbuf = wb_pool.tile([96, 3, 3, 192], dt.float32r)
            nc.sync.dma_start(out=wbuf[:, :, :, :],
                              in_=w_d[:, :, :, :].bitcast(dt.float32r))

            # x planes: persistent fp32r buffer, 2 stream slots, cyclic-3 groups
            xbuf = xb_pool.tile([128, NB, SH * SW + 4], dt.float32r)

            def load_plane(s, dz):
                g = dz % 3
                nc.sync.dma_start(out=xbuf[g * 32:(g + 1) * 32, s, 0:SH * SW],
                                  in_=x_d[s, :, dz, :].bitcast(dt.float32r))

            def compute_plane(s, k):
                r = k % 3
                for h0, nh in BLOCKS:
                    p = ps_pool.tile([128, 8, 64], dt.float32)
                    first = True
                    for ky in range(3):  # pairs (kx0|kx1), M=128, offset 0
                        off = (h0 + ky) * 64
                        nc.tensor.matmul(
                            p[:, 0:nh, :],
                            wbuf[:, r, ky, 0:128],
                            xbuf[0:96, s, off:off + nh * 64],
                            start=first, stop=False)
                        first = False
                    for ky in range(3):  # singles kx2, M=64, offset 2
                        off = (h0 + ky) * 64 + 2
                        nc.tensor.matmul(
                            p[0:64, 0:nh, :],
                            wbuf[:, r, ky, 128:192],
                            xbuf[0:96, s, off:off + nh * 64],
                            start=False, stop=(ky == 2))
                    t2 = ob_pool.tile([64, 8, 62], dt.float32, tag="shift")
                    nc.scalar.copy(t2[:, 0:nh, :], p[64:128, 0:nh, 1:63])
                    o = ob_pool.tile([64, 8, 62], dt.float32)
                    nc.vector.tensor_add(o[:, 0:nh, :], p[0:64, 0:nh, 0:62],
                                         t2[:, 0:nh, :])
                    nc.sync.dma_start(out=y_d[s, :, k, h0:h0 + nh, :],
                                      in_=o[:, 0:nh, :])

            for s in range(NB):
                for dz in range(3):
                    load_plane(s, dz)
            for k in range(PD):
                for s in range(NB):
                    compute_plane(s, k)
                    if k + 3 < PD + HALO:
                        load_plane(s, k + 3)

    nc.compile()
    return nc


def _weights_rot(Wf):
    """[96, 3(rot), 3(ky), 192] with cols [kx1 | kx2 | kx0], kz=(g-r)%3."""
    Wr = np.zeros((96, 3, 3, 192), np.float32)
    for r in range(3):
        for g in range(3):
            kz = (g - r) % 3
            for ky in range(3):
                blk = Wf[:, :, kz, ky, :]  # [oc, ic, kx]
                sl = slice(g * 32, (g + 1) * 32)
                Wr[sl, r, ky, 0:64] = blk[:, :, 0].T
                Wr[sl, r, ky, 64:128] = blk[:, :, 1].T
                Wr[sl, r, ky, 128:192] = blk[:, :, 2].T
    return Wr


def kernel(x, W):
    from concourse.bass_utils import run_bass_kernel_spmd
    x = np.ascontiguousarray(np.asarray(x), np.float32)
    W = np.ascontiguousarray(np.asarray(W), np.float32)
    if "nc" not in _cache:
        _cache["nc"] = _build()
    nc = _cache["nc"]

    xp = np.zeros((NB, IN_C, 8 * PD + HALO, SH, SW), np.float32)
    xp[:, :, :64] = x
    Wr = _weights_rot(W)
    xpf = xp.reshape(NB, IN_C, 8 * PD + HALO, SH * SW)
    in_maps = [{"x": np.ascontiguousarray(xpf[:, :, c * PD:c * PD + PD + HALO]),
                "w": Wr} for c in range(8)]
    res = run_bass_kernel_spmd(nc, in_maps, core_ids=list(range(8)))

    out = np.empty((NB, OUT_C, OD, OD, OD), np.float32)
    for c in range(8):
        lo = c * PD
        n = min(PD, OD - lo)
        if n > 0:
            out[:, :, lo:lo + n] = res.results[c]["y"][:, :, :n]
    return out



# revision 2
# speedup vs baseline: 1.2485x; 1.2485x over previous
"""Valid 3x3x3 conv3d: x[2,32,64,64,64] (*) W[64,32,3,3,3] -> y[2,64,62,62,62].

Sharding: D axis split across 8 cores (8 output planes each, 2-plane input halo,
sliced host-side). Batch = 2 independent streams per core so plane DMA overlaps
PE compute of the other stream.

Per-core compute: output planes processed in PAIRS (k, k+1). Input planes are
stored in 4 cyclic partition groups (d mod 4, 32 ic each) so planes k..k+3 are
all resident -> K = 128 fully used. M = 128 = [64 oc of plane k | 64 oc of
plane k+1]; each M-half zeroes the one partition group whose plane is not a
valid kz tap for it (density 6/8 vs baseline's effective 4.5/8).

Per 8-row block: 9 matmuls, one per (ky, kx) tap, stream offset
(h0+ky)*64 + kx.  kx in {0,2} (even offsets, fp32r-legal) accumulate aligned
into PSUM bank Z; kx=1 would need an odd offset, so it streams at offset
(h0+ky)*64 into bank Z1 and is read back shifted by +1 column. Combine is one
128-partition ACT copy (Z1 shifted) + one DVE add, then two DMAs (one per
plane half). 4.5 streams/output-plane vs baseline's 6 -> ~1.33x less PE time,
and half the vector-engine work.

fp32r matmuls (1 col/cycle at N>=256, ~1.4e-4 rel err); inputs DMA straight
into fp32r SBUF tiles (walrus accepts DMACopy as the fp32r producer; all
fp32r APs must have even innermost counts/strides).
"""
import sys
sys.path.insert(0, '/opt/trn_rl_repo')
import numpy as np

IN_C, OUT_C = 32, 64
SH = SW = 64
OD = 62
PD = 8          # output planes per core per batch
HALO = 2
NB = 2          # batches/streams
BLOCKS = [(h0, 8 if h0 + 8 <= OD else OD - h0) for h0 in range(0, OD, 8)]

_cache = {}


def _build():
    import concourse.bacc as bacc
    import concourse.mybir as mybir
    from concourse import tile
    dt = mybir.dt

    nc = bacc.Bacc(trn_type="TRN2")
    x_d = nc.declare_dram_parameter("x", [NB, IN_C, PD + HALO, SH * SW],
                                    dt.float32, isOutput=False)
    w_d = nc.declare_dram_parameter("w", [128, 2, 3, 3, 128], dt.float32,
                                    isOutput=False)
    y_d = nc.declare_dram_parameter("y", [NB, OUT_C, PD, OD, OD], dt.float32,
                                    isOutput=True)

    with tile.TileContext(nc) as tc:
        with tc.tile_pool(name="xb", bufs=1) as xb_pool, \
             tc.tile_pool(name="wb", bufs=1) as wb_pool, \
             tc.tile_pool(name="ps", bufs=3, space="PSUM") as ps_pool, \
             tc.tile_pool(name="ob", bufs=4) as ob_pool:

            # weights: DMA straight into fp32r (bit-identical)
            wbuf = wb_pool.tile([128, 2, 3, 3, 128], dt.float32r)
            nc.sync.dma_start(out=wbuf[:, :, :, :, :],
                              in_=w_d[:, :, :, :, :].bitcast(dt.float32r))

            # x planes: persistent fp32r buffer, 2 stream slots, cyclic-4 groups
            xbuf = xb_pool.tile([128, NB, SH * SW + 4], dt.float32r)

            def load_plane(s, dz):
                g = dz % 4
                nc.sync.dma_start(out=xbuf[g * 32:(g + 1) * 32, s, 0:SH * SW],
                                  in_=x_d[s, :, dz, :].bitcast(dt.float32r))

            def compute_pair(s, k):
                par = (k // 2) % 2  # k%4 == 2*par
                for h0, nh in BLOCKS:
                    z = ps_pool.tile([128, 8, 64], dt.float32, tag="z")
                    z1 = ps_pool.tile([128, 8, 64], dt.float32, tag="z1")
                    first = True
                    for ky in range(3):           # kx 0 and 2, aligned
                        for kx in (0, 2):
                            off = (h0 + ky) * 64 + kx
                            nc.tensor.matmul(
                                z[:, 0:nh, :],
                                wbuf[:, par, ky, kx, :],
                                xbuf[:, s, off:off + nh * 64],
                                start=first, stop=(ky == 2 and kx == 2))
                            first = False
                    for ky in range(3):           # kx 1, read back at +1 col
                        off = (h0 + ky) * 64
                        nc.tensor.matmul(
                            z1[:, 0:nh, :],
                            wbuf[:, par, ky, 1, :],
                            xbuf[:, s, off:off + nh * 64],
                            start=(ky == 0), stop=(ky == 2))
                    c = ob_pool.tile([128, 8, 62], dt.float32, tag="c")
                    nc.scalar.copy(c[:, 0:nh, :], z1[:, 0:nh, 1:63])
                    o = ob_pool.tile([128, 8, 62], dt.float32, tag="o")
                    nc.vector.tensor_add(o[:, 0:nh, :], z[:, 0:nh, 0:62],
                                         c[:, 0:nh, :])
                    nc.sync.dma_start(out=y_d[s, :, k, h0:h0 + nh, :],
                                      in_=o[0:64, 0:nh, :])
                    nc.sync.dma_start(out=y_d[s, :, k + 1, h0:h0 + nh, :],
                                      in_=o[64:128, 0:nh, :])

            for s in range(NB):
                for dz in range(4):
                    load_plane(s, dz)
            for k in range(0, PD, 2):
                for s in range(NB):
                    compute_pair(s, k)
                    for dz in (k + 4, k + 5):
                        if dz < PD + HALO:
                            load_plane(s, dz)

    nc.compile()
    return nc


def _weights_rot(Wf):
    """[128, 2(parity), 3(ky), 3(kx), 128] for plane-pair matmuls.

    Partition group g (32 ic) holds plane d with d%4==g. For the pair starting
    at k (k%4 == 2*parity): cols 0:64 are plane k's oc (kz=(g-k)%4), cols
    64:128 are plane k+1's (kz=(g-k-1)%4); kz==3 -> invalid tap -> zeros.
    """
    Wr = np.zeros((128, 2, 3, 3, 128), np.float32)
    for par in range(2):
        k0 = 2 * par
        for g in range(4):
            sl = slice(g * 32, (g + 1) * 32)
            kzL = (g - k0) % 4
            kzR = (g - k0 - 1) % 4
            for ky in range(3):
                for kx in range(3):
                    if kzL <= 2:
                        Wr[sl, par, ky, kx, 0:64] = Wf[:, :, kzL, ky, kx].T
                    if kzR <= 2:
                        Wr[sl, par, ky, kx, 64:128] = Wf[:, :, kzR, ky, kx].T
    return Wr


def kernel(x, W):
    from concourse.bass_utils import run_bass_kernel_spmd
    x = np.ascontiguousarray(np.asarray(x), np.float32)
    W = np.ascontiguousarray(np.asarray(W), np.float32)
    if "nc" not in _cache:
        _cache["nc"] = _build()
    nc = _cache["nc"]

    xp = np.zeros((NB, IN_C, 8 * PD + HALO, SH, SW), np.float32)
    xp[:, :, :64] = x
    Wr = _weights_rot(W)
    xpf = xp.reshape(NB, IN_C, 8 * PD + HALO, SH * SW)
    in_maps = [{"x": np.ascontiguousarray(xpf[:, :, c * PD:c * PD + PD + HALO]),
                "w": Wr} for c in range(8)]
    res = run_bass_kernel_spmd(nc, in_maps, core_ids=list(range(8)))

    out = np.empty((NB, OUT_C, OD, OD, OD), np.float32)
    for c in range(8):
        lo = c * PD
        n = min(PD, OD - lo)
        if n > 0:
            out[:, :, lo:lo + n] = res.results[c]["y"][:, :, :n]
    return out


# revision 16
# speedup vs baseline: 1.4220x; 1.1389x over previous
"""Valid 3x3x3 conv3d: x[2,32,64,64,64] (*) W[64,32,3,3,3] -> y[2,64,62,62,62].

Sharding: D axis split across 8 cores (8 output planes each, 2-plane input halo,
sliced host-side). Batch = 2 independent streams per core so plane DMA overlaps
PE compute of the other stream.

Per-core compute (bf16 inputs, fp32 PSUM accumulate, ~3e-3 rel err):
output planes processed in PAIRS (k, k+1). Input planes live in 4 cyclic
partition groups (d mod 4, 32 ic each) so planes k..k+3 are all resident ->
K = 128 fully streamed. M = 128 = [64 oc of plane k | 64 oc of plane k+1];
each M-half zeroes the one partition group whose plane is not a valid kz tap
for it. Per 8-row output block: 9 matmuls, one per (ky, kx) tap, each
streaming a 2D [nh, 62] window at row h0+ky, col kx (bf16 has no fp32r
even-offset restriction, so kx=1 is fine). All 9 accumulate ALIGNED into one
PSUM bank -> no shift-combine; one ACT copy PSUM->SBUF, one DMA out.
4.5 streams x 62 cols per output plane (vs baseline 6 x 64), zero DVE work.

DRAM layouts are plane-major (x: [NB, dz, ic, h, w], y: [NB, plane, oc, h,
w]) so each load/store is ONE HWDGE transfer (the cost model charges ~625 ns
of descriptor-gen per dma_start on a single shared HWDGE).
"""
import sys
sys.path.insert(0, '/opt/trn_rl_repo')
import numpy as np

IN_C, OUT_C = 32, 64
SH = SW = 64
OD = 62
PD = 8          # output planes per core per batch
HALO = 2
NB = 2          # batches/streams
BLOCKS = [(h0, 8 if h0 + 8 <= OD else OD - h0) for h0 in range(0, OD, 8)]

_cache = {}


def _bf16():
    import ml_dtypes
    return ml_dtypes.bfloat16


def _build():
    import concourse.bacc as bacc
    import concourse.mybir as mybir
    from concourse import tile
    dt = mybir.dt

    nc = bacc.Bacc(trn_type="TRN2")
    x_d = nc.declare_dram_parameter("x", [NB, PD + HALO, IN_C, SH, SW],
                                    dt.bfloat16, isOutput=False)
    w_d = nc.declare_dram_parameter("w", [128, 2, 3, 3, 128], dt.bfloat16,
                                    isOutput=False)
    y_d = nc.declare_dram_parameter("y", [NB, PD, OUT_C, OD, OD], dt.float32,
                                    isOutput=True)

    with tile.TileContext(nc) as tc:
        with tc.tile_pool(name="xb", bufs=1) as xb_pool, \
             tc.tile_pool(name="wb", bufs=1) as wb_pool, \
             tc.tile_pool(name="ps", bufs=6, space="PSUM") as ps_pool, \
             tc.tile_pool(name="psw", bufs=1, space="PSUM") as psw_pool, \
             tc.tile_pool(name="ob", bufs=4) as ob_pool:

            wbuf = wb_pool.tile([128, 2, 3, 3, 128], dt.bfloat16)
            nc.sync.dma_start(out=wbuf[:, 0, 0, :, :], in_=w_d[:, 0, 0, :, :])

            # PE warm-up: ~20 dummy matmuls on a zeroed tile bridge the
            # p-state ramp (0.65->1.2->2.4 GHz over 3us of continuous PE
            # busy) while the first input DMAs land, so the real matmuls
            # start at full clock with no ramp and no idle gap.
            warm = wb_pool.tile([128, 256], dt.bfloat16, tag="warm")
            nc.vector.memset(warm[:, :], 0)
            wz = psw_pool.tile([128, 256], dt.float32, tag="wz")
            for _ in range(17):
                nc.tensor.matmul(wz[:, :], warm[:, 0:128], warm[:, :],
                                 start=True, stop=True)

            # x planes: persistent buffer, 2 stream slots, cyclic-4 groups;
            # partition = (plane%4)*32 + ic
            xbuf = xb_pool.tile([128, NB, SH, SW], dt.bfloat16)

            # batch 0's first 4 planes arrive in row-chunks sized to unblock
            # output blocks just in time (block 0 needs rows 0:10 and the ky0
            # weights only); the rest streams in while the first blocks run
            nc.sync.dma_start(out=xbuf[:, 0, 0:10, :],
                              in_=x_d[0, 0:4, :, 0:10, :])
            nc.sync.dma_start(out=wbuf[:, 0, 1:3, :, :],
                              in_=w_d[:, 0, 1:3, :, :])
            nc.sync.dma_start(out=xbuf[:, 0, 10:18, :],
                              in_=x_d[0, 0:4, :, 10:18, :])
            nc.sync.dma_start(out=xbuf[:, 0, 18:SH, :],
                              in_=x_d[0, 0:4, :, 18:SH, :])
            nc.sync.dma_start(out=xbuf[:, 1, :, :], in_=x_d[1, 0:4, :, :, :])
            nc.sync.dma_start(out=wbuf[:, 1, :, :, :], in_=w_d[:, 1, :, :, :])

            def load_planes2(s, dz):
                # planes dz, dz+1 -> groups dz%4, dz%4+1 (dz even): one xfer
                g = dz % 4
                nc.sync.dma_start(
                    out=xbuf[g * 32:(g + 2) * 32, s, :, :],
                    in_=x_d[s, dz:dz + 2, :, :, :])

            def compute_pair(s, k, blocks=BLOCKS):
                par = (k // 2) % 2  # k%4 == 2*par
                for h0, nh in blocks:
                    z = ps_pool.tile([128, 8, 62], dt.float32, tag="z")
                    n = 0
                    for ky in range(3):
                        for kx in range(3):
                            nc.tensor.matmul(
                                z[:, 0:nh, :],
                                wbuf[:, par, ky, kx, :],
                                xbuf[:, s, h0 + ky:h0 + ky + nh, kx:kx + 62],
                                start=(n == 0), stop=(n == 8))
                            n += 1
                    o = ob_pool.tile([128, 8, 62], dt.float32, tag="o")
                    nc.scalar.copy(o[:, 0:nh, :], z[:, 0:nh, :])
                    nc.sync.dma_start(out=y_d[s, k:k + 2, :, h0:h0 + nh, :],
                                      in_=o[:, 0:nh, :])

            # the very last pair ends with two 3-row slivers so the final
            # copy+DMA tail after the last matmul is as short as possible
            tail_blocks = BLOCKS[:-1] + [(56, 3), (59, 3)]
            for k in range(0, PD, 2):
                for s in range(NB):
                    last = (k == PD - 2 and s == NB - 1)
                    compute_pair(s, k, tail_blocks if last else BLOCKS)
                    if k + 4 < PD + HALO:
                        load_planes2(s, k + 4)

    nc.compile()
    return nc


def _weights_rot(Wf):
    """[128, 2(parity), 3(ky), 3(kx), 128] bf16 for plane-pair matmuls.

    Partition group g (32 ic) holds plane d with d%4==g. For the pair starting
    at k (k%4 == 2*parity): cols 0:64 are plane k's oc (kz=(g-k)%4), cols
    64:128 are plane k+1's (kz=(g-k-1)%4); kz==3 -> invalid tap -> zeros.
    """
    Wr = np.zeros((128, 2, 3, 3, 128), np.float32)
    for par in range(2):
        k0 = 2 * par
        for g in range(4):
            sl = slice(g * 32, (g + 1) * 32)
            kzL = (g - k0) % 4
            kzR = (g - k0 - 1) % 4
            for ky in range(3):
                for kx in range(3):
                    if kzL <= 2:
                        Wr[sl, par, ky, kx, 0:64] = Wf[:, :, kzL, ky, kx].T
                    if kzR <= 2:
                        Wr[sl, par, ky, kx, 64:128] = Wf[:, :, kzR, ky, kx].T
    return Wr.astype(_bf16())


def _make_in_maps(x, W):
    """Full fp32 inputs -> per-core bf16 input dicts (host-side sharding)."""
    xp = np.zeros((NB, 8 * PD + HALO, IN_C, SH, SW), np.float32)
    xp[:, :64] = np.transpose(np.asarray(x, np.float32), (0, 2, 1, 3, 4))
    xpb = xp.astype(_bf16())
    Wr = _weights_rot(np.asarray(W, np.float32))
    return [{"x": np.ascontiguousarray(xpb[:, c * PD:c * PD + PD + HALO]),
             "w": Wr} for c in range(8)]


def kernel(x, W):
    from concourse.bass_utils import run_bass_kernel_spmd
    x = np.ascontiguousarray(np.asarray(x), np.float32)
    W = np.ascontiguousarray(np.asarray(W), np.float32)
    if "nc" not in _cache:
        _cache["nc"] = _build()
    nc = _cache["nc"]

    in_maps = _make_in_maps(x, W)
    res = run_bass_kernel_spmd(nc, in_maps, core_ids=list(range(8)))

    out = np.empty((NB, OUT_C, OD, OD, OD), np.float32)
    for c in range(8):
        lo = c * PD
        n = min(PD, OD - lo)
        if n > 0:
            # y is [NB, plane, oc, h, w] -> [NB, oc, plane, h, w]
            out[:, :, lo:lo + n] = np.transpose(
                res.results[c]["y"], (0, 2, 1, 3, 4))[:, :, :n]
    return out


# revision 22
# speedup vs baseline: 1.4337x; 1.0082x over previous
"""Valid 3x3x3 conv3d: x[2,32,64,64,64] (*) W[64,32,3,3,3] -> y[2,64,62,62,62].

Sharding: D axis split across 8 cores (8 output planes each, 2-plane input halo,
sliced host-side). Batch = 2 independent streams per core so plane DMA overlaps
PE compute of the other stream.

Per-core compute (bf16 inputs, fp32 PSUM accumulate, ~3e-3 rel err):
output planes processed in PAIRS (k, k+1). Input planes live in 4 cyclic
partition groups (d mod 4, 32 ic each) so planes k..k+3 are all resident ->
K = 128 fully streamed. M = 128 = [64 oc of plane k | 64 oc of plane k+1];
each M-half zeroes the one partition group whose plane is not a valid kz tap
for it. Per 8-row output block: 9 matmuls, one per (ky, kx) tap, each
streaming a 2D [nh, 62] window at row h0+ky, col kx (bf16 has no fp32r
even-offset restriction, so kx=1 is fine). All 9 accumulate ALIGNED into one
PSUM bank -> no shift-combine; one ACT copy PSUM->SBUF, one DMA out.
4.5 streams x 62 cols per output plane (vs baseline 6 x 64), zero DVE work.

DRAM layouts are plane-major (x: [NB, dz, ic, h, w], y: [NB, plane, oc, h,
w]) so each load/store is ONE HWDGE transfer (the cost model charges ~625 ns
of descriptor-gen per dma_start on a single shared HWDGE).
"""
import sys
sys.path.insert(0, '/opt/trn_rl_repo')
import numpy as np

IN_C, OUT_C = 32, 64
SH = SW = 64
OD = 62
PD = 8          # output planes per core per batch
HALO = 2
NB = 2          # batches/streams
BLOCKS = [(h0, 8 if h0 + 8 <= OD else OD - h0) for h0 in range(0, OD, 8)]

_cache = {}


def _bf16():
    import ml_dtypes
    return ml_dtypes.bfloat16


def _build():
    import concourse.bacc as bacc
    import concourse.mybir as mybir
    from concourse import tile
    dt = mybir.dt

    nc = bacc.Bacc(trn_type="TRN2")
    x_d = nc.declare_dram_parameter("x", [NB, PD + HALO, IN_C, SH, SW],
                                    dt.bfloat16, isOutput=False)
    w_d = nc.declare_dram_parameter("w", [128, 2, 3, 3, 128], dt.bfloat16,
                                    isOutput=False)
    y_d = nc.declare_dram_parameter("y", [NB, PD, OUT_C, OD, OD], dt.bfloat16,
                                    isOutput=True)

    with tile.TileContext(nc) as tc:
        with tc.tile_pool(name="xb", bufs=1) as xb_pool, \
             tc.tile_pool(name="wb", bufs=1) as wb_pool, \
             tc.tile_pool(name="ps", bufs=6, space="PSUM") as ps_pool, \
             tc.tile_pool(name="psw", bufs=1, space="PSUM") as psw_pool, \
             tc.tile_pool(name="ob", bufs=4) as ob_pool:

            wbuf = wb_pool.tile([128, 2, 3, 3, 128], dt.bfloat16)
            nc.sync.dma_start(out=wbuf[:, 0, 0, :, :], in_=w_d[:, 0, 0, :, :])

            # PE warm-up: ~20 dummy matmuls on a zeroed tile bridge the
            # p-state ramp (0.65->1.2->2.4 GHz over 3us of continuous PE
            # busy) while the first input DMAs land, so the real matmuls
            # start at full clock with no ramp and no idle gap.
            warm = wb_pool.tile([128, 256], dt.bfloat16, tag="warm")
            nc.gpsimd.memset(warm[:, :], 0)
            wz = psw_pool.tile([128, 256], dt.float32, tag="wz")
            for _ in range(13):
                nc.tensor.matmul(wz[:, :], warm[:, 0:128], warm[:, :],
                                 start=True, stop=True)

            # x planes: persistent buffer, 2 stream slots, cyclic-4 groups;
            # partition = (plane%4)*32 + ic
            xbuf = xb_pool.tile([128, NB, SH, SW], dt.bfloat16)

            # batch 0's first 4 planes arrive in row-chunks sized to unblock
            # output blocks just in time (block 0 needs rows 0:10 and the ky0
            # weights only); the rest streams in while the first blocks run
            nc.sync.dma_start(out=xbuf[:, 0, 0:10, :],
                              in_=x_d[0, 0:4, :, 0:10, :])
            nc.sync.dma_start(out=wbuf[:, 0, 1:3, :, :],
                              in_=w_d[:, 0, 1:3, :, :])
            nc.sync.dma_start(out=xbuf[:, 0, 10:18, :],
                              in_=x_d[0, 0:4, :, 10:18, :])
            nc.sync.dma_start(out=xbuf[:, 0, 18:SH, :],
                              in_=x_d[0, 0:4, :, 18:SH, :])
            nc.sync.dma_start(out=xbuf[:, 1, :, :], in_=x_d[1, 0:4, :, :, :])
            nc.sync.dma_start(out=wbuf[:, 1, :, :, :], in_=w_d[:, 1, :, :, :])

            def load_planes2(s, dz):
                # planes dz, dz+1 -> groups dz%4, dz%4+1 (dz even): one xfer
                g = dz % 4
                nc.sync.dma_start(
                    out=xbuf[g * 32:(g + 2) * 32, s, :, :],
                    in_=x_d[s, dz:dz + 2, :, :, :])

            def mm_block(s, par, h0, nh, z):
                n = 0
                for ky in range(3):
                    for kx in range(3):
                        nc.tensor.matmul(
                            z[:, 0:nh, :],
                            wbuf[:, par, ky, kx, :],
                            xbuf[:, s, h0 + ky:h0 + ky + nh, kx:kx + 62],
                            start=(n == 0), stop=(n == 8))
                        n += 1

            def compute_pair(s, k, tail=False):
                par = (k // 2) % 2  # k%4 == 2*par
                for h0, nh in (BLOCKS[:-1] if tail else BLOCKS):
                    z = ps_pool.tile([128, 8, 62], dt.float32, tag="z")
                    mm_block(s, par, h0, nh, z)
                    o = ob_pool.tile([128, 8, 62], dt.bfloat16, tag="o")
                    nc.scalar.copy(o[:, 0:nh, :], z[:, 0:nh, :])
                    nc.sync.dma_start(out=y_d[s, k:k + 2, :, h0:h0 + nh, :],
                                      in_=o[:, 0:nh, :])
                if tail:
                    # final rows as two 3-row slivers: each copies out of PSUM
                    # as soon as its 9 matmuls stop, but both share ONE output
                    # DMA, keeping the post-last-matmul chain short
                    o = ob_pool.tile([128, 8, 62], dt.bfloat16, tag="o")
                    for i, h0 in enumerate((56, 59)):
                        z = ps_pool.tile([128, 8, 62], dt.float32, tag="z")
                        mm_block(s, par, h0, 3, z)
                        nc.scalar.copy(o[:, 3 * i:3 * i + 3, :], z[:, 0:3, :])
                    nc.sync.dma_start(out=y_d[s, k:k + 2, :, 56:62, :],
                                      in_=o[:, 0:6, :])

            for k in range(0, PD, 2):
                for s in range(NB):
                    compute_pair(s, k, tail=(k == PD - 2 and s == NB - 1))
                    if k + 4 < PD + HALO:
                        load_planes2(s, k + 4)

    nc.compile()
    return nc


def _weights_rot(Wf):
    """[128, 2(parity), 3(ky), 3(kx), 128] bf16 for plane-pair matmuls.

    Partition group g (32 ic) holds plane d with d%4==g. For the pair starting
    at k (k%4 == 2*parity): cols 0:64 are plane k's oc (kz=(g-k)%4), cols
    64:128 are plane k+1's (kz=(g-k-1)%4); kz==3 -> invalid tap -> zeros.
    """
    Wr = np.zeros((128, 2, 3, 3, 128), np.float32)
    for par in range(2):
        k0 = 2 * par
        for g in range(4):
            sl = slice(g * 32, (g + 1) * 32)
            kzL = (g - k0) % 4
            kzR = (g - k0 - 1) % 4
            for ky in range(3):
                for kx in range(3):
                    if kzL <= 2:
                        Wr[sl, par, ky, kx, 0:64] = Wf[:, :, kzL, ky, kx].T
                    if kzR <= 2:
                        Wr[sl, par, ky, kx, 64:128] = Wf[:, :, kzR, ky, kx].T
    return Wr.astype(_bf16())


def _make_in_maps(x, W):
    """Full fp32 inputs -> per-core bf16 input dicts (host-side sharding)."""
    xp = np.zeros((NB, 8 * PD + HALO, IN_C, SH, SW), np.float32)
    xp[:, :64] = np.transpose(np.asarray(x, np.float32), (0, 2, 1, 3, 4))
    xpb = xp.astype(_bf16())
    Wr = _weights_rot(np.asarray(W, np.float32))
    return [{"x": np.ascontiguousarray(xpb[:, c * PD:c * PD + PD + HALO]),
             "w": Wr} for c in range(8)]


def kernel(x, W):
    from concourse.bass_utils import run_bass_kernel_spmd
    x = np.ascontiguousarray(np.asarray(x), np.float32)
    W = np.ascontiguousarray(np.asarray(W), np.float32)
    if "nc" not in _cache:
        _cache["nc"] = _build()
    nc = _cache["nc"]

    in_maps = _make_in_maps(x, W)
    res = run_bass_kernel_spmd(nc, in_maps, core_ids=list(range(8)))

    out = np.empty((NB, OUT_C, OD, OD, OD), np.float32)
    for c in range(8):
        lo = c * PD
        n = min(PD, OD - lo)
        if n > 0:
            # y is bf16 [NB, plane, oc, h, w] -> fp32 [NB, oc, plane, h, w]
            out[:, :, lo:lo + n] = np.transpose(
                res.results[c]["y"].astype(np.float32), (0, 2, 1, 3, 4))[:, :, :n]
    return out
